# revision 18
# baseline (speedup 1.0000x reference)
"""Distributed Trainium2 (8 NeuronCore) kernel for a 2-layer GCN diffusion
denoiser: out = GCN2(relu(LN(GCN1(h + t_emb)))).

Sharding: nodes (and their incident edges) are sharded across the 8 cores by
contiguous dst ranges.  Each core computes x@W for its shard (bf16), the
per-conv feature tables are AllGathered, edge source rows are fetched with
dma_gather (256B bf16 rows), and the scatter-add aggregation is a chain of
one-hot matmuls on the TensorEngine accumulated in PSUM per 128-dst window
(the self-loop term is a diagonal one-hot against the core-local table).
All floating-point math runs on device; the host only preprocesses the
integer edge structure (partitioning, degree coefficients, schedules).
"""

import os
import sys
from contextlib import ExitStack

if "/opt/trn_rl_repo" not in sys.path:
    sys.path.insert(0, "/opt/trn_rl_repo")

import numpy as np
import ml_dtypes

import concourse.bacc as bacc
import concourse.bass as bass
import concourse.mybir as mybir
from concourse.bass_utils import run_bass_kernel_spmd
from concourse.library_config import mlp

BF16 = ml_dtypes.bfloat16
F32 = mybir.dt.float32
BF = mybir.dt.bfloat16
I16 = mybir.dt.int16
Alu = mybir.AluOpType
Act = mybir.ActivationFunctionType
AxisX = mybir.AxisListType.X

N_NODES = 100000
C = 128
N_CORES = 8
NBUK = 4

NPS = 4           # rotating PSUM banks for aggregation runs
NOH = 12          # one-hot ring size
NGB = 2           # gather-buffer slots
CALL_CHUNKS = 32  # edge-chunks per dma_gather call
RING = 4          # h2 staging / h2T ring depth


# ---------------------------------------------------------------------------
# host-side schedule (pure integer graph preprocessing)
# ---------------------------------------------------------------------------

class _Run:
    __slots__ = ("q", "w", "nch", "has_diag", "ec0")

    def __init__(self, q, w, nch, has_diag, ec0):
        self.q, self.w, self.nch, self.has_diag, self.ec0 = q, w, nch, has_diag, ec0


class _Item:
    __slots__ = ("kind", "w", "run", "start", "stop", "ec")

    def __init__(self, kind, w, run, start, stop, ec):
        self.kind, self.w, self.run = kind, w, run
        self.start, self.stop, self.ec = start, stop, ec


def _make_schedule(src, dst, coef, shard):
    K = N_CORES
    npad = shard * K
    nt = shard // 128
    bukrows = npad // NBUK
    assert bukrows <= 32767 and npad % NBUK == 0 and shard % 128 == 0

    counts = np.zeros((K, NBUK, nt), np.int64)
    per_core_edges = []
    for k in range(K):
        m = (dst >= k * shard) & (dst < (k + 1) * shard)
        s, d, c = src[m], dst[m] - k * shard, coef[m]
        q = s // bukrows
        w = d // 128
        order = np.lexsort((w, q))
        s, d, c, q, w = s[order], d[order], c[order], q[order], w[order]
        np.add.at(counts[k], (q, w), 1)
        per_core_edges.append((s, d, c, q, w))

    nch_qw = -(-counts.max(axis=0) // 128)          # shared ceil-div chunk counts

    first_q = {}
    for q in range(NBUK):
        for w in range(nt):
            if nch_qw[q, w] and w not in first_q:
                first_q[w] = q

    runs = []
    for w in range(nt):
        if w not in first_q:
            runs.append(_Run(-1, w, 0, True, None))
    ec = 0
    for q in range(NBUK):
        for w in range(nt):
            if nch_qw[q, w]:
                runs.append(_Run(q, w, int(nch_qw[q, w]), first_q[w] == q, ec))
                ec += int(nch_qw[q, w])
    nech = ec
    nrun = len(runs)

    items = []
    item0 = []                      # first item index of each run
    calls = []                      # (q, ec0, nchunks)
    ec_call = {}
    for r, run in enumerate(runs):
        item0.append(len(items))
        nit = (1 if run.has_diag else 0) + run.nch
        j = 0
        if run.has_diag:
            items.append(_Item("diag", run.w, r, True, nit == 1, None))
            j += 1
        for jj in range(run.nch):
            items.append(_Item("edge", run.w, r, j == 0, j == nit - 1, run.ec0 + jj))
            j += 1
        for jj in range(run.nch):
            e = run.ec0 + jj
            if calls and calls[-1][0] == run.q and \
                    calls[-1][1] + calls[-1][2] == e and calls[-1][2] < CALL_CHUNKS:
                calls[-1] = (calls[-1][0], calls[-1][1], calls[-1][2] + 1)
            else:
                calls.append((run.q, e, 1))
            ec_call[e] = len(calls) - 1
    nitem = len(items)
    ncall = len(calls)

    item_run = [it.run for it in items]
    run_last_call = {r: ec_call[run.ec0 + run.nch - 1]
                     for r, run in enumerate(runs) if run.nch}
    last_item_of_call = {}
    for i, it in enumerate(items):
        if it.kind == "edge":
            last_item_of_call[ec_call[it.ec]] = i

    last_run_of_win = {}
    for r, run in enumerate(runs):
        last_run_of_win[run.w] = r
    ep_order = sorted(range(nt), key=lambda w: last_run_of_win[w])
    ep_pos = {w: e for e, w in enumerate(ep_order)}

    # per-core padded edge-data arrays, indexed by global edge-chunk position
    core_arrays = []
    for k in range(K):
        s, d, c, q, w = per_core_edges[k]
        idxl = np.zeros(max(nech, 1) * 128, np.int16)
        dstl = np.zeros(max(nech, 1) * 128, np.float32)
        cf = np.zeros(max(nech, 1) * 128, np.float32)
        keys = q.astype(np.int64) * nt + w.astype(np.int64)
        for run in runs:
            if run.q < 0:
                continue
            key = run.q * nt + run.w
            lo = np.searchsorted(keys, key, "left")
            hi = np.searchsorted(keys, key, "right")
            n = hi - lo
            base = run.ec0 * 128
            idxl[base:base + n] = (s[lo:hi] - run.q * bukrows).astype(np.int16)
            dstl[base:base + n] = (d[lo:hi] % 128).astype(np.float32)
            cf[base:base + n] = c[lo:hi]
        gidx = np.tile(idxl.reshape(-1, 16).T, (8, 1)).copy()      # [128, nech*8]
        dst2d = dstl.reshape(-1, 128).T.copy()                     # [128, nech]
        cf2d = cf.reshape(-1, 128).T.copy()
        core_arrays.append((gidx, dst2d, cf2d))

    return dict(npad=npad, nt=nt, bukrows=bukrows, runs=runs, items=items,
                item0=item0, item_run=item_run, calls=calls, ec_call=ec_call,
                nech=nech, nrun=nrun, nitem=nitem, ncall=ncall,
                run_last_call=run_last_call, last_item_of_call=last_item_of_call,
                last_run_of_win=last_run_of_win, ep_order=ep_order, ep_pos=ep_pos,
                core_arrays=core_arrays)


# ---------------------------------------------------------------------------
# bass program
# ---------------------------------------------------------------------------

class _Waits:
    """Per-(engine, sem) monotone tracker; emits wait_ge only when it rises."""

    def __init__(self):
        self.seen = {}

    def __call__(self, eng, s, val):
        key = (id(eng), id(s))
        if val > self.seen.get(key, 0):
            eng.wait_ge(s, val)
            self.seen[key] = val


def _build(S, shard):
    FAKE_GATHER = bool(os.environ.get("BASS_FAKE_GATHER"))
    FAKE_CC = bool(os.environ.get("BASS_FAKE_CC"))
    CCI = 16 if FAKE_CC else 1
    PHASE = int(os.environ.get("BASS_PHASE", "5"))
    EP_NO_TTR = bool(os.environ.get("EP_NO_TTR"))
    EP_NO_RECIP = bool(os.environ.get("EP_NO_RECIP"))
    EP_NO_SQRT = bool(os.environ.get("EP_NO_SQRT"))
    EP_NO_RELU = bool(os.environ.get("EP_NO_RELU"))  # 1=phaseA 2=+agg 3=+epi 4=+Aprime 5=full
    nt, npad, bukrows = S["nt"], S["npad"], S["bukrows"]
    runs, items, calls = S["runs"], S["items"], S["calls"]
    item0, item_run, ec_call = S["item0"], S["item_run"], S["ec_call"]
    nech, nrun, nitem, ncall = S["nech"], S["nrun"], S["nitem"], S["ncall"]
    run_last_call = S["run_last_call"]
    last_item_of_call = S["last_item_of_call"]
    last_run_of_win = S["last_run_of_win"]
    ep_order, ep_pos = S["ep_order"], S["ep_pos"]
    necw = max(nech, 1)

    nc = bacc.Bacc("TRN2", detect_race_conditions=not os.environ.get("BASS_NO_RACE"))

    din = lambda n, sh, dt: nc.declare_dram_parameter(n, sh, dt, isOutput=False)
    h_sT_d = din("h_sT", [128, shard], BF)
    gidx_d = din("gidx", [128, necw * 8], I16)
    dst_d = din("dst2d", [128, necw], F32)
    cf_d = din("coef2d", [128, necw], F32)
    dv2_d = din("dinv2col", [128, nt], F32)
    W1b_d = din("W1b", [128, 128], BF)
    W2b_d = din("W2b", [128, 128], BF)
    W1f_d = din("W1f", [128, 128], F32)
    tW2f_d = din("tW2f", [128, 128], F32)
    tW1c_d = din("tW1col", [128, 1], F32)
    tb1c_d = din("tb1col", [128, 1], F32)
    tb2c_d = din("tb2col", [128, 1], F32)
    tcol_d = din("tcol", [128, 1], F32)
    b1r_d = din("b1rep", [128, 128], F32)
    b2r_d = din("b2rep", [128, 128], F32)
    lnwr_d = din("lnwrep", [128, 128], BF)
    lnbr_d = din("lnbrep", [128, 128], BF)
    iota_d = din("iota", [128, 128], BF)
    linc_d = din("lincol", [128, 1], F32)
    eps_d = din("epscol", [128, 1], F32)
    idm_d = din("idmat", [128, 128], BF)
    ones_d = din("onesrow", [1, 128], BF)
    out_d = nc.declare_dram_parameter("out_shard", [shard, 128], F32, isOutput=True)

    ag1_in = nc.dram_tensor("ag1_in", [shard, 128], BF)
    table1 = nc.dram_tensor("table1", [npad, 128], BF, addr_space="Shared")
    ag2_in = nc.dram_tensor("ag2_in", [shard, 128], BF)
    table2 = nc.dram_tensor("table2", [npad, 128], BF, addr_space="Shared")
    tables = [table1, table2]
    ag_ins = [ag1_in, ag2_in]

    with ExitStack() as ctx:
        sbuf = lambda n, sh, dt: ctx.enter_context(nc.sbuf_tensor(n, sh, dt))
        psum = lambda n, sh, dt=F32: ctx.enter_context(nc.psum_tensor(n, sh, dt))
        sem = lambda n: ctx.enter_context(nc.semaphore(n))

        hsT = sbuf("hsT", [128, shard], BF)
        gidx = sbuf("gidx_sb", [128, necw * 8], I16)
        dst2d = sbuf("dst2d_sb", [128, necw], F32)
        cf2d = sbuf("cf2d_sb", [128, necw], F32)
        dv2 = sbuf("dv2_sb", [128, nt], F32)
        W1b = sbuf("W1b_sb", [128, 128], BF)
        W2b = sbuf("W2b_sb", [128, 128], BF)
        W1f = sbuf("W1f_sb", [128, 128], F32)
        tW2f = sbuf("tW2f_sb", [128, 128], F32)
        tW1c = sbuf("tW1c_sb", [128, 1], F32)
        tb1c = sbuf("tb1c_sb", [128, 1], F32)
        tb2c = sbuf("tb2c_sb", [128, 1], F32)
        tcol = sbuf("tcol_sb", [128, 1], F32)
        b1r = sbuf("b1r_sb", [128, 128], F32)
        b2r = sbuf("b2r_sb", [128, 128], F32)
        lnwr = sbuf("lnwr_sb", [128, 128], BF)
        lnbr = sbuf("lnbr_sb", [128, 128], BF)
        iota = sbuf("iota_sb", [128, 128], BF)
        linc = sbuf("linc_sb", [128, 1], F32)
        epsc = sbuf("eps_sb", [128, 1], F32)
        idmat = sbuf("idmat_sb", [128, 128], BF)
        onesr = sbuf("ones_sb", [1, 128], BF)

        xw1 = sbuf("xw1", [128, shard], BF)
        xw2 = sbuf("xw2", [128, shard], BF)
        agg = sbuf("agg", [128, shard], F32)
        gb = sbuf("gb", [128, NGB, CALL_CHUNKS * 128], BF)
        oh = sbuf("oh", [128, NOH, 128], BF)
        h2full = sbuf("h2full", [128, shard], BF)
        h2T = sbuf("h2T", [128, RING, 128], BF)
        c16 = sbuf("c16", [128, 2, 128], BF)
        sqscr = sbuf("sqscr", [128, 128], F32)
        ucol = sbuf("ucol", [128, 1], F32)
        vcol = sbuf("vcol", [128, 1], F32)
        r1bf = sbuf("r1bf", [1, 128], BF)
        stat = sbuf("stat", [128, 8], F32)

        ps_run = [psum(f"ps_run{i}", [128, 128]) for i in range(NPS)]
        ps_pa = [psum("ps_pa0", [128, 128]), psum("ps_pa1", [128, 128])]
        ps_tr = [psum("ps_tr0", [128, 128], BF), psum("ps_tr1", [128, 128], BF)]

        s_ld = sem("s_ld")
        s_tb1 = sem("s_tb1")
        s_tb2 = sem("s_tb2")
        s_cc = sem("s_cc")
        s_gat = sem("s_gat")
        s_pe_tr = sem("s_pe_tr")
        s_dv_trc = sem("s_dv_trc")
        s_out = sem("s_out")
        s_pe_run = sem("s_pe_run")
        s_pe_pa = sem("s_pe_pa")
        s_pe_pa2 = sem("s_pe_pa2")
        s_pe_tm = sem("s_pe_tm")
        s_dv_oh = sem("s_dv_oh")
        s_dv_drain = sem("s_dv_drain")
        s_dv_pa = sem("s_dv_pa")
        s_dv_pa2 = sem("s_dv_pa2")
        s_dv_tm = sem("s_dv_tm")
        s_dv_ep = sem("s_dv_ep")
        s_ac = sem("s_ac")
        s_ac_h2 = sem("s_ac_h2")

        wt = _Waits()
        N_LOADS = 22

        with nc.Block() as block:

            # ---------------- SYNC ----------------
            @block.sync
            def _(sync):
                loads = [
                    (hsT, h_sT_d), (gidx, gidx_d), (dst2d, dst_d), (cf2d, cf_d),
                    (dv2, dv2_d), (W1b, W1b_d), (W2b, W2b_d), (W1f, W1f_d),
                    (tW2f, tW2f_d), (tW1c, tW1c_d), (tb1c, tb1c_d),
                    (tb2c, tb2c_d), (tcol, tcol_d), (b1r, b1r_d), (b2r, b2r_d),
                    (lnwr, lnwr_d), (lnbr, lnbr_d), (iota, iota_d),
                    (linc, linc_d), (epsc, eps_d), (idmat, idm_d), (onesr, ones_d),
                ]
                assert len(loads) == N_LOADS
                for dst_t, src_t in loads:
                    sync.dma_start(dst_t[:, :], src_t[:, :]).then_inc(s_ld, 16)

                # conv1 phase-A table writes
                for t in range(nt):
                    wt(sync, s_dv_pa, t + 1)
                    sync.dma_start(ag1_in[t * 128:(t + 1) * 128, :],
                                   xw1[:, t * 128:(t + 1) * 128]).then_inc(s_tb1, 16)

                # conv2 table writes (epilogue order)
                if PHASE >= 4:
                    for e, w in enumerate(ep_order):
                        wt(sync, s_dv_pa2, e + 1)
                        sync.dma_start(ag2_in[w * 128:(w + 1) * 128, :],
                                       xw2[:, w * 128:(w + 1) * 128]).then_inc(
                                           s_tb2, 16)

                # output writes
                for w in range(nt):
                    if PHASE == 1:
                        wt(sync, s_dv_pa, nt)
                    elif PHASE in (2, 3, 4):
                        wt(sync, s_dv_drain, last_run_of_win[w] + 1)
                    else:
                        wt(sync, s_dv_drain, nrun + last_run_of_win[w] + 1)
                    sync.dma_start(out_d[w * 128:(w + 1) * 128, :],
                                   agg[:, w * 128:(w + 1) * 128]).then_inc(s_out, 16)
                wt(sync, s_out, 16 * nt)

            # ---------------- GPSIMD ----------------
            @block.gpsimd
            def _(gpsimd):
                gpsimd.load_library(mlp)
                NCONV = 0 if PHASE == 1 else (1 if PHASE <= 4 else 2)
                for conv in range(NCONV):
                    wt(gpsimd, s_tb1 if conv == 0 else s_tb2, 16 * nt)
                    if FAKE_CC:
                        gpsimd.dma_start(tables[conv][0:shard, :],
                                         ag_ins[conv][:, :]).then_inc(s_cc, CCI)
                    else:
                        gpsimd.collective_compute(
                            "AllGather", Alu.bypass,
                            replica_groups=[list(range(N_CORES))],
                            ins=[ag_ins[conv].ap().opt()],
                            outs=[tables[conv].ap().opt()]).then_inc(s_cc, CCI)
                    wt(gpsimd, s_cc, CCI * (conv + 1))
                    for g, (q, ec0, nchk) in enumerate(calls):
                        gg = conv * ncall + g
                        if gg >= NGB:
                            gp = gg - NGB
                            rel = (gp // ncall) * nrun + \
                                item_run[last_item_of_call[gp % ncall]] + 1
                            wt(gpsimd, s_pe_run, rel)
                        nidx = nchk * 128
                        dstap = gb[:, gg % NGB, 0:nchk * 128].rearrange(
                            "p (n e) -> p n e", e=128)
                        if FAKE_GATHER:
                            src = tables[conv][0:nchk * 128, :].rearrange(
                                "(a p) c -> p a c", p=128)
                            gpsimd.dma_start(dstap, src).then_inc(s_gat, 16)
                        else:
                            gpsimd.dma_gather(
                                dstap, tables[conv][q * bukrows:(q + 1) * bukrows, :],
                                gidx[:, ec0 * 8:(ec0 + nchk) * 8], nidx, nidx, 128,
                                single_packet=False,
                            ).then_inc(s_gat, 16)
                wt(gpsimd, s_gat, 16 * NCONV * ncall)

            # ---------------- TENSOR ----------------
            @block.tensor
            def _(tensor):
                wt(tensor, s_ld, 16 * N_LOADS)
                # t-MLP
                wt(tensor, s_dv_tm, 1)
                tensor.matmul(ps_pa[0][:, 0:1], tW2f[:, :], ucol[:, :],
                              start=True, stop=True).then_inc(s_pe_tm, 1)
                wt(tensor, s_dv_tm, 2)
                tensor.matmul(ps_pa[1][0:1, 0:128], vcol[:, :], W1f[:, :],
                              start=True, stop=True).then_inc(s_pe_tm, 1)
                wt(tensor, s_dv_tm, 3)
                # conv1 phase A
                for t in range(nt):
                    if t >= 2:
                        wt(tensor, s_dv_pa, t - 1)
                    p = ps_pa[t % 2]
                    tensor.matmul(p[:, :], hsT[:, t * 128:(t + 1) * 128], W1b[:, :],
                                  start=True, stop=False)
                    tensor.matmul(p[:, :], onesr[:, :], r1bf[:, :],
                                  start=False, stop=True).then_inc(s_pe_pa, 1)
                # aggregation + (for conv1) phase A'
                for conv in range(0 if PHASE == 1 else 1 if PHASE <= 4 else 2):
                    wt(tensor, s_dv_pa if conv == 0 else s_dv_pa2, nt)
                    xw = xw1 if conv == 0 else xw2
                    for r, run in enumerate(runs):
                        R = conv * nrun + r
                        if R >= NPS:
                            wt(tensor, s_dv_drain, R - NPS + 1)
                        nit = (1 if run.has_diag else 0) + run.nch
                        wt(tensor, s_dv_oh, conv * nitem + item0[r] + nit)
                        if run.nch:
                            wt(tensor, s_gat,
                               16 * (conv * ncall + run_last_call[r] + 1))
                        for j in range(nit):
                            i = item0[r] + j
                            it = items[i]
                            lhs = oh[:, (conv * nitem + i) % NOH, :]
                            if it.kind == "diag":
                                rhs = xw[:, it.w * 128:(it.w + 1) * 128]
                            else:
                                g = ec_call[it.ec]
                                off = it.ec - calls[g][1]
                                rhs = gb[:, (conv * ncall + g) % NGB,
                                         off * 128:(off + 1) * 128]
                            mm = tensor.matmul(ps_run[R % NPS][:, :], lhs, rhs,
                                               start=it.start, stop=it.stop)
                            if it.stop:
                                mm.then_inc(s_pe_run, 1)
                    if conv == 0 and PHASE >= 4:
                        def a2_mm(e):
                            wt(tensor, s_dv_trc, e + 1)
                            if e >= 2:
                                wt(tensor, s_dv_pa2, e - 1)
                            tensor.matmul(ps_pa[e % 2][:, :], h2T[:, e % RING, :],
                                          W2b[:, :], start=True,
                                          stop=True).then_inc(s_pe_pa2, 1)

                        for e in range(nt):
                            w = ep_order[e]
                            wt(tensor, s_ac_h2, e + 1)
                            if e >= 2:
                                wt(tensor, s_dv_trc, e - 1)
                            tensor.transpose(ps_tr[e % 2][:, :],
                                             h2full[:, w * 128:(w + 1) * 128],
                                             idmat[:, :]).then_inc(s_pe_tr, 1)
                            if e >= 1:
                                a2_mm(e - 1)
                        if nt >= 1:
                            a2_mm(nt - 1)

            # ---------------- VECTOR ----------------
            ep_state = dict(ep=0)

            def emit_epilogue(vector, w):
                e = ep_state["ep"]
                aggw = agg[:, w * 128:(w + 1) * 128]
                ssum, ssq = stat[:, 0:1], stat[:, 1:2]
                smu, ssmu = stat[:, 2:3], stat[:, 3:4]
                svarn, ssd, srstd = stat[:, 4:5], stat[:, 5:6], stat[:, 6:7]
                vector.drain()
                vector.tensor_reduce(ssum, aggw, AxisX, Alu.add)
                vector.tensor_mul(sqscr[:, :], aggw, aggw)
                vector.drain()
                vector.tensor_reduce(ssq, sqscr[:, :], AxisX, Alu.add)
                vector.drain()
                vector.tensor_scalar(smu, ssum, 1.0 / 128.0, None, Alu.mult)
                vector.drain()
                vector.tensor_scalar(ssmu, ssum, smu, None, Alu.mult)
                vector.drain()
                vector.tensor_scalar(svarn, ssq, ssmu, 1.0 / 128.0,
                                     Alu.subtract, Alu.mult).then_inc(s_dv_ep, 1)
                wt(vector, s_ac, e + 1)          # ACT produced sd
                if EP_NO_RECIP:
                    vector.tensor_copy(srstd, ssd)
                else:
                    vector.reciprocal(srstd, ssd)
                vector.drain()
                if e >= 2:
                    wt(vector, s_ac_h2, e - 1)   # c16 slot free
                cw = c16[:, e % 2, :]
                vector.tensor_scalar(cw, aggw, smu, srstd, Alu.subtract, Alu.mult)
                vector.drain()
                vector.tensor_mul(cw, cw, lnwr[:, :])
                vector.drain()
                vector.tensor_add(cw, cw, lnbr[:, :]).then_inc(s_dv_ep, 1)
                ep_state["ep"] = e + 1

            def emit_drain(vector, D, conv, win_left):
                r = D - conv * nrun
                run = runs[r]
                aggw = agg[:, run.w * 128:(run.w + 1) * 128]
                wt(vector, s_pe_run, D + 1)
                vector.tensor_add(aggw, aggw, ps_run[D % NPS][:, :]).then_inc(
                    s_dv_drain, 1)
                win_left[run.w] -= 1
                if win_left[run.w] == 0 and conv == 0 and PHASE >= 3:
                    emit_epilogue(vector, run.w)

            @block.vector
            def _(vector):
                wt(vector, s_ld, 16 * N_LOADS)
                # t-MLP
                vector.tensor_scalar(ucol[:, :], tW1c[:, :], tcol[:, :], tb1c[:, :],
                                     Alu.mult, Alu.add)
                vector.drain()
                vector.tensor_relu(ucol[:, :], ucol[:, :]).then_inc(s_dv_tm, 1)
                wt(vector, s_pe_tm, 1)
                vector.tensor_add(vcol[:, :], ps_pa[0][:, 0:1],
                                  tb2c[:, :]).then_inc(s_dv_tm, 1)
                wt(vector, s_pe_tm, 2)
                vector.tensor_copy(r1bf[:, :],
                                   ps_pa[1][0:1, 0:128]).then_inc(s_dv_tm, 1)
                # conv1 phase-A PSUM -> SBUF (bf16)
                for t in range(nt):
                    wt(vector, s_pe_pa, t + 1)
                    vector.tensor_copy(xw1[:, t * 128:(t + 1) * 128],
                                       ps_pa[t % 2][:, :]).then_inc(s_dv_pa, 1)
                # agg init: broadcast b1 row-tile across windows
                for t in range(nt):
                    vector.tensor_copy(agg[:, t * 128:(t + 1) * 128], b1r[:, :])

                for conv in range(0 if PHASE == 1 else 1 if PHASE <= 4 else 2):
                    win_left = {}
                    for run in runs:
                        win_left[run.w] = win_left.get(run.w, 0) + 1
                    pend = []
                    for r, run in enumerate(runs):
                        R = conv * nrun + r
                        nit = (1 if run.has_diag else 0) + run.nch
                        for j in range(nit):
                            i = conv * nitem + item0[r] + j
                            it = items[item0[r] + j]
                            if i >= NOH:
                                ii = i - NOH
                                blk = (ii // nitem) * nrun + item_run[ii % nitem]
                                wt(vector, s_pe_run, blk + 1)
                            if it.kind == "diag":
                                s1, s2 = linc[:, :], dv2[:, it.w:it.w + 1]
                            else:
                                s1 = dst2d[:, it.ec:it.ec + 1]
                                s2 = cf2d[:, it.ec:it.ec + 1]
                            vector.tensor_scalar(oh[:, i % NOH, :], iota[:, :],
                                                 s1, s2, Alu.is_equal,
                                                 Alu.mult).then_inc(s_dv_oh, 1)
                        pend.append(R)
                        while pend and pend[0] <= R - 1:
                            emit_drain(vector, pend.pop(0), conv, win_left)
                    while pend:
                        emit_drain(vector, pend.pop(0), conv, win_left)
                    if conv == 0 and PHASE >= 4:
                        # h2T copies + conv2 phase-A' copies, then agg re-init
                        def pa2_copy(e):
                            w = ep_order[e]
                            wt(vector, s_pe_pa2, e + 1)
                            vector.tensor_copy(xw2[:, w * 128:(w + 1) * 128],
                                               ps_pa[e % 2][:, :]).then_inc(
                                                   s_dv_pa2, 1)

                        for e in range(nt):
                            wt(vector, s_pe_tr, e + 1)
                            if e >= RING:
                                wt(vector, s_pe_pa2, e - RING + 1)
                            vector.tensor_copy(h2T[:, e % RING, :],
                                               ps_tr[e % 2][:, :]).then_inc(
                                                   s_dv_trc, 1)
                            if e >= 1:
                                pa2_copy(e - 1)
                        if nt >= 1:
                            pa2_copy(nt - 1)
                        if PHASE >= 5:
                            for t in range(nt):
                                vector.tensor_copy(agg[:, t * 128:(t + 1) * 128],
                                                   b2r[:, :])

            # ---------------- SCALAR (ACT) ----------------
            @block.scalar
            def _(scalar):
                wt(scalar, s_ld, 16 * N_LOADS)
                for e in range(nt if PHASE >= 3 else 0):
                    wt(scalar, s_dv_ep, 2 * e + 1)
                    if EP_NO_SQRT:
                        scalar.activation(stat[:, 5:6], stat[:, 4:5],
                                          Act.Copy).then_inc(s_ac, 1)
                    else:
                        scalar.activation(stat[:, 5:6], stat[:, 4:5], Act.Sqrt,
                                          bias=epsc[:, :]).then_inc(s_ac, 1)
                    wt(scalar, s_dv_ep, 2 * e + 2)
                    w = ep_order[e]
                    scalar.activation(h2full[:, w * 128:(w + 1) * 128],
                                      c16[:, e % 2, :],
                                      Act.Copy if EP_NO_RELU else Act.Relu).then_inc(
                                          s_ac_h2, 1)

        nc.compile()
    return nc


# ---------------------------------------------------------------------------
# top level
# ---------------------------------------------------------------------------

LAST_NC = None


def _run_problem(h_noisy, edge_index, t, tW1, tb1, tW2, tb2, W1, b1, W2, b2,
                 ln_w, ln_b, n_nodes, shard, trace_dir=None):
    K = N_CORES
    npad = shard * K
    src = np.asarray(edge_index[0], np.int64)
    dst = np.asarray(edge_index[1], np.int64)

    deg = (np.bincount(dst, minlength=n_nodes).astype(np.float32) + 1.0)
    dinv = (1.0 / np.sqrt(deg)).astype(np.float32)
    coef = (dinv[src] * dinv[dst]).astype(np.float32)
    dinv2 = (dinv * dinv).astype(np.float32)
    dinv2_pad = np.ones(npad, np.float32)
    dinv2_pad[:n_nodes] = dinv2

    S = _make_schedule(src, dst, coef, shard)
    nt = S["nt"]

    h_pad = np.zeros((npad, C), np.float32)
    h_pad[:n_nodes] = np.asarray(h_noisy, np.float32)

    shared = {
        "W1b": np.asarray(W1, np.float32).astype(BF16),
        "W2b": np.asarray(W2, np.float32).astype(BF16),
        "W1f": np.asarray(W1, np.float32),
        "tW2f": np.asarray(tW2, np.float32),
        "tW1col": np.asarray(tW1, np.float32).reshape(C, 1),
        "tb1col": np.asarray(tb1, np.float32).reshape(C, 1),
        "tb2col": np.asarray(tb2, np.float32).reshape(C, 1),
        "tcol": np.full((C, 1), np.float32(np.asarray(t).reshape(-1)[0]), np.float32),
        "b1rep": np.tile(np.asarray(b1, np.float32).reshape(1, C), (128, 1)),
        "b2rep": np.tile(np.asarray(b2, np.float32).reshape(1, C), (128, 1)),
        "lnwrep": np.tile(np.asarray(ln_w, np.float32).reshape(1, C),
                          (128, 1)).astype(BF16),
        "lnbrep": np.tile(np.asarray(ln_b, np.float32).reshape(1, C),
                          (128, 1)).astype(BF16),
        "iota": np.tile(np.arange(128, dtype=np.float32), (128, 1)).astype(BF16),
        "lincol": np.arange(128, dtype=np.float32).reshape(128, 1),
        "epscol": np.full((128, 1), 1e-5, np.float32),
        "idmat": np.eye(128, dtype=np.float32).astype(BF16),
        "onesrow": np.ones((1, 128), np.float32).astype(BF16),
    }

    in_maps = []
    for k in range(K):
        gidx, dst2d, cf2d = S["core_arrays"][k]
        hs = h_pad[k * shard:(k + 1) * shard].astype(BF16)
        dv2col = np.zeros((128, nt), np.float32)
        for w in range(nt):
            dv2col[:, w] = dinv2_pad[k * shard + w * 128: k * shard + (w + 1) * 128]
        m = dict(shared)
        m["h_sT"] = np.ascontiguousarray(hs.T)
        m["gidx"] = gidx
        m["dst2d"] = dst2d
        m["coef2d"] = cf2d
        m["dinv2col"] = dv2col
        in_maps.append(m)

    nc = _build(S, shard)
    global LAST_NC
    LAST_NC = nc

    if trace_dir is not None:
        res = _run_traced(nc, in_maps, trace_dir)
    else:
        res = run_bass_kernel_spmd(nc, in_maps, list(range(K)))

    out = np.concatenate([res.results[k]["out_shard"] for k in range(K)], axis=0)
    return out[:n_nodes].astype(np.float32)


def _run_traced(nc, in_maps, trace_dir):
    """Run with NRT/NTFF profiling via the axon ctypes hook (test harness)."""
    import types
    import antenv
    if "antenv.axon_hooks" not in sys.modules:
        mod = types.ModuleType("antenv.axon_hooks")
        mod._hook = None
        mod.set_axon_ntff_profile_hook = lambda h: setattr(mod, "_hook", h)
        mod.get_axon_ntff_profile_hook = lambda: mod._hook
        sys.modules["antenv.axon_hooks"] = mod
        antenv.axon_hooks = mod
    from trn_agent_boot.trn_boot import _ntff_profile_via_ctypes
    hook = _ntff_profile_via_ctypes("/opt/axon/libaxon_pjrt.so")
    os.makedirs(trace_dir, exist_ok=True)
    with hook(trace_dir, [0]):
        res = run_bass_kernel_spmd(nc, in_maps, list(range(N_CORES)))
    return res


def kernel(h_noisy, edge_index, t, tW1, tb1, tW2, tb2, W1, b1, W2, b2,
           ln_w, ln_b):
    trace_dir = os.environ.get("BASS_KERNEL_TRACE_DIR") or None
    return _run_problem(
        np.asarray(h_noisy), np.asarray(edge_index), np.asarray(t),
        np.asarray(tW1), np.asarray(tb1), np.asarray(tW2), np.asarray(tb2),
        np.asarray(W1), np.asarray(b1), np.asarray(W2), np.asarray(b2),
        np.asarray(ln_w), np.asarray(ln_b),
        n_nodes=N_NODES, shard=12544, trace_dir=trace_dir)


# revision 19
# speedup vs baseline: 1.9928x; 1.9928x over previous
"""Distributed Trainium2 (8 NeuronCore) kernel for a 2-layer GCN diffusion
denoiser: out = GCN2(relu(LN(GCN1(h + t_emb)))).

Sharding: nodes (and their incident edges) are sharded across the 8 cores by
contiguous dst ranges.  Each core computes x@W for its shard (bf16), the
per-conv feature tables are AllGathered, edge source rows are fetched with
dma_gather on 4 parallel SWDGE queues (one per source-range bucket), and the
scatter-add aggregation is a chain of one-hot matmuls on the TensorEngine
accumulated in PSUM per 128-dst window (the self-loop term is a diagonal
one-hot against the core-local table).  All floating-point math runs on
device; the host only preprocesses the integer edge structure.
"""

import os
import sys
from contextlib import ExitStack

if "/opt/trn_rl_repo" not in sys.path:
    sys.path.insert(0, "/opt/trn_rl_repo")

import numpy as np
import ml_dtypes

import concourse.bacc as bacc
import concourse.bass as bass
import concourse.mybir as mybir
from concourse.bass_utils import run_bass_kernel_spmd
from concourse.library_config import mlp

BF16 = ml_dtypes.bfloat16
F32 = mybir.dt.float32
BF = mybir.dt.bfloat16
I16 = mybir.dt.int16
Alu = mybir.AluOpType
Act = mybir.ActivationFunctionType
AxisX = mybir.AxisListType.X

N_NODES = 100000
C = 128
N_CORES = 8
NBUK = 4

NPS = 4           # rotating PSUM banks for per-window accumulation
NGBQ = 3          # gather-buffer slots per bucket queue
CALL_CHUNKS = 8   # edge-chunks per dma_gather call (<=1024 idxs: HW limit)


# ---------------------------------------------------------------------------
# host-side schedule (pure integer graph preprocessing)
# ---------------------------------------------------------------------------

class _Item:
    __slots__ = ("kind", "w", "q", "pos", "ec", "start", "stop")

    def __init__(self, kind, w, q, pos, ec, start, stop):
        self.kind, self.w, self.q, self.pos, self.ec = kind, w, q, pos, ec
        self.start, self.stop = start, stop


def _make_schedule(src, dst, coef, shard):
    K = N_CORES
    npad = shard * K
    nt = shard // 128
    bukrows = npad // NBUK
    assert bukrows <= 32767 and npad % NBUK == 0 and shard % 128 == 0

    counts = np.zeros((K, NBUK, nt), np.int64)
    per_core_edges = []
    for k in range(K):
        m = (dst >= k * shard) & (dst < (k + 1) * shard)
        s, d, c = src[m], dst[m] - k * shard, coef[m]
        q = s // bukrows
        w = d // 128
        order = np.lexsort((w, q))
        s, d, c, q, w = s[order], d[order], c[order], q[order], w[order]
        np.add.at(counts[k], (q, w), 1)
        per_core_edges.append((s, d, c, q, w))

    nch_qw = -(-counts.max(axis=0) // 128)     # [NBUK, nt] shared chunk counts

    # bucket-major chunk positions: bucket q's stream is its (q, w) runs in
    # ascending w; global ec = buk_base[q] + within-bucket position.
    nchq = nch_qw.sum(axis=1)                  # chunks per bucket
    buk_base = np.zeros(NBUK + 1, np.int64)
    buk_base[1:] = np.cumsum(nchq)
    nech = int(buk_base[-1])
    chunk_pos = {}                             # (q, w) -> within-bucket pos
    for q in range(NBUK):
        p = 0
        for w in range(nt):
            if nch_qw[q, w]:
                chunk_pos[(q, w)] = p
                p += int(nch_qw[q, w])

    # window-major item stream (diag first, then each bucket's chunks)
    items = []
    win_item0 = []
    for w in range(nt):
        win_item0.append(len(items))
        its = [_Item("diag", w, -1, -1, -1, False, False)]
        for q in range(NBUK):
            for j in range(int(nch_qw[q, w])):
                pos = chunk_pos[(q, w)] + j
                its.append(_Item("edge", w, q, pos,
                                 int(buk_base[q]) + pos, False, False))
        its[0].start = True
        its[-1].stop = True
        items.extend(its)
    nitem = len(items)

    # gather calls: per bucket, CALL_CHUNKS chunks per call
    ncall_q = [int(-(-nchq[q] // CALL_CHUNKS)) if nchq[q] else 0
               for q in range(NBUK)]
    call_sizes = [[int(min(CALL_CHUNKS, int(nchq[q]) - j * CALL_CHUNKS))
                   for j in range(ncall_q[q])] for q in range(NBUK)]
    # window containing the last chunk of each (q, call): for WAR release
    pos_to_win = [dict() for _ in range(NBUK)]
    for (q, w), p0 in chunk_pos.items():
        for j in range(int(nch_qw[q, w])):
            pos_to_win[q][p0 + j] = w
    call_rel_win = []
    for q in range(NBUK):
        rels = []
        for j in range(ncall_q[q]):
            last = min((j + 1) * CALL_CHUNKS, int(nchq[q])) - 1
            rels.append(pos_to_win[q][last])
        call_rel_win.append(rels)

    if nt > 1:
        max_items_win = max(
            (win_item0[i + 1] if i + 1 < nt else nitem) - win_item0[i]
            for i in range(nt))
    else:
        max_items_win = nitem

    # per-core padded edge-data arrays, indexed by global ec
    core_arrays = []
    for k in range(K):
        s, d, c, q, w = per_core_edges[k]
        idxl = np.zeros(max(nech, 1) * 128, np.int16)
        dstl = np.zeros(max(nech, 1) * 128, np.float32)
        cf = np.zeros(max(nech, 1) * 128, np.float32)
        keys = q.astype(np.int64) * nt + w.astype(np.int64)
        for (qq, ww), p0 in chunk_pos.items():
            key = qq * nt + ww
            lo = np.searchsorted(keys, key, "left")
            hi = np.searchsorted(keys, key, "right")
            n = hi - lo
            base = (int(buk_base[qq]) + p0) * 128
            idxl[base:base + n] = (s[lo:hi] - qq * bukrows).astype(np.int16)
            dstl[base:base + n] = (d[lo:hi] % 128).astype(np.float32)
            cf[base:base + n] = c[lo:hi]
        gidx = np.tile(idxl.reshape(-1, 16).T, (8, 1)).copy()
        dst2d = dstl.reshape(-1, 128).T.copy()
        cf2d = cf.reshape(-1, 128).T.copy()
        core_arrays.append((gidx, dst2d, cf2d))

    return dict(npad=npad, nt=nt, bukrows=bukrows, items=items,
                win_item0=win_item0, nitem=nitem, nech=nech,
                buk_base=[int(x) for x in buk_base],
                ncall_q=ncall_q, call_sizes=call_sizes,
                call_rel_win=call_rel_win, max_items_win=int(max_items_win),
                core_arrays=core_arrays)


# ---------------------------------------------------------------------------
# bass program
# ---------------------------------------------------------------------------

class _Waits:
    """Per-(engine, sem) monotone tracker; emits wait_ge only when it rises."""

    def __init__(self):
        self.seen = {}

    def __call__(self, eng, s, val):
        key = (id(eng), id(s))
        if val > self.seen.get(key, 0):
            eng.wait_ge(s, val)
            self.seen[key] = val


def _build(S, shard):
    nt, npad, bukrows = S["nt"], S["npad"], S["bukrows"]
    items, win_item0, nitem = S["items"], S["win_item0"], S["nitem"]
    nech, buk_base = S["nech"], S["buk_base"]
    ncall_q, call_sizes = S["ncall_q"], S["call_sizes"]
    call_rel_win = S["call_rel_win"]
    necw = max(nech, 1)
    NOH = min(64, max(16, 2 * S["max_items_win"]))
    PHASE = int(os.environ.get("BASS_PHASE", "5"))

    nc = bacc.Bacc("TRN2", num_swdge_queues=NBUK,
                   detect_race_conditions=not os.environ.get("BASS_NO_RACE"))

    din = lambda n, sh, dt: nc.declare_dram_parameter(n, sh, dt, isOutput=False)
    h_sT_d = din("h_sT", [128, shard], BF)
    gidx_d = din("gidx", [128, necw * 8], I16)
    dst_d = din("dst2d", [128, necw], F32)
    cf_d = din("coef2d", [128, necw], F32)
    dv2_d = din("dinv2col", [128, nt], F32)
    W1b_d = din("W1b", [128, 128], BF)
    W2b_d = din("W2b", [128, 128], BF)
    W1f_d = din("W1f", [128, 128], F32)
    tW2f_d = din("tW2f", [128, 128], F32)
    tW1c_d = din("tW1col", [128, 1], F32)
    tb1c_d = din("tb1col", [128, 1], F32)
    tb2c_d = din("tb2col", [128, 1], F32)
    tcol_d = din("tcol", [128, 1], F32)
    b1r_d = din("b1rep", [128, 128], F32)
    b2r_d = din("b2rep", [128, 128], F32)
    lnwr_d = din("lnwrep", [128, 128], BF)
    lnbr_d = din("lnbrep", [128, 128], BF)
    iota_d = din("iota", [128, 128], BF)
    linc_d = din("lincol", [128, 1], F32)
    eps_d = din("epscol", [128, 1], F32)
    idm_d = din("idmat", [128, 128], BF)
    ones_d = din("onesrow", [1, 128], BF)
    out_d = nc.declare_dram_parameter("out_shard", [shard, 128], F32, isOutput=True)

    ag1_in = nc.dram_tensor("ag1_in", [shard, 128], BF)
    table1 = nc.dram_tensor("table1", [npad, 128], BF, addr_space="Shared")
    ag2_in = nc.dram_tensor("ag2_in", [shard, 128], BF)
    table2 = nc.dram_tensor("table2", [npad, 128], BF, addr_space="Shared")
    tables = [table1, table2]
    ag_ins = [ag1_in, ag2_in]

    with ExitStack() as ctx:
        sbuf = lambda n, sh, dt: ctx.enter_context(nc.sbuf_tensor(n, sh, dt))
        psum = lambda n, sh, dt=F32: ctx.enter_context(nc.psum_tensor(n, sh, dt))
        sem = lambda n: ctx.enter_context(nc.semaphore(n))

        hsT = sbuf("hsT", [128, shard], BF)
        gidx = sbuf("gidx_sb", [128, necw * 8], I16)
        dst2d = sbuf("dst2d_sb", [128, necw], F32)
        cf2d = sbuf("cf2d_sb", [128, necw], F32)
        dv2 = sbuf("dv2_sb", [128, nt], F32)
        W1b = sbuf("W1b_sb", [128, 128], BF)
        W2b = sbuf("W2b_sb", [128, 128], BF)
        W1f = sbuf("W1f_sb", [128, 128], F32)
        tW2f = sbuf("tW2f_sb", [128, 128], F32)
        tW1c = sbuf("tW1c_sb", [128, 1], F32)
        tb1c = sbuf("tb1c_sb", [128, 1], F32)
        tb2c = sbuf("tb2c_sb", [128, 1], F32)
        tcol = sbuf("tcol_sb", [128, 1], F32)
        b1r = sbuf("b1r_sb", [128, 128], F32)
        b2r = sbuf("b2r_sb", [128, 128], F32)
        lnwr = sbuf("lnwr_sb", [128, 128], BF)
        lnbr = sbuf("lnbr_sb", [128, 128], BF)
        iota = sbuf("iota_sb", [128, 128], BF)
        linc = sbuf("linc_sb", [128, 1], F32)
        epsc = sbuf("eps_sb", [128, 1], F32)
        idmat = sbuf("idmat_sb", [128, 128], BF)
        onesr = sbuf("ones_sb", [1, 128], BF)

        xw1 = sbuf("xw1", [128, shard], BF)
        xw2 = sbuf("xw2", [128, shard], BF)
        agg = sbuf("agg", [128, shard], F32)
        h2full = sbuf("h2full", [128, shard], BF)
        gb = sbuf("gb", [128, NBUK, NGBQ, CALL_CHUNKS * 128], BF)
        oh = sbuf("oh", [128, NOH, 128], BF)
        h2T = sbuf("h2T", [128, 4, 128], BF)
        c16 = sbuf("c16", [128, 2, 128], BF)
        sqscr = sbuf("sqscr", [128, 128], F32)
        ucol = sbuf("ucol", [128, 1], F32)
        vcol = sbuf("vcol", [128, 1], F32)
        r1bf = sbuf("r1bf", [1, 128], BF)
        stat = sbuf("stat", [128, 8], F32)

        ps_run = [psum(f"ps_run{i}", [128, 128]) for i in range(NPS)]
        ps_pa = [psum("ps_pa0", [128, 128]), psum("ps_pa1", [128, 128])]
        ps_tr = [psum("ps_tr0", [128, 128], BF), psum("ps_tr1", [128, 128], BF)]

        s_ld = sem("s_ld")
        s_tb1 = sem("s_tb1")
        s_tb2 = sem("s_tb2")
        s_cc = sem("s_cc")
        s_gq = [sem(f"s_gq{q}") for q in range(NBUK)]
        s_out = sem("s_out")
        s_pe_run = sem("s_pe_run")     # one inc per completed window
        s_pe_pa = sem("s_pe_pa")
        s_pe_pa2 = sem("s_pe_pa2")
        s_pe_tm = sem("s_pe_tm")
        s_pe_tr = sem("s_pe_tr")
        s_dv_oh = sem("s_dv_oh")
        s_dv_drain = sem("s_dv_drain")  # one inc per drained window
        s_dv_pa = sem("s_dv_pa")
        s_dv_pa2 = sem("s_dv_pa2")
        s_dv_tm = sem("s_dv_tm")
        s_dv_trc = sem("s_dv_trc")
        s_dv_ep = sem("s_dv_ep")
        s_ac = sem("s_ac")
        s_ac_h2 = sem("s_ac_h2")

        wt = _Waits()
        N_LOADS = 22

        with nc.Block() as block:

            # ---------------- SYNC ----------------
            @block.sync
            def _(sync):
                loads = [
                    (hsT, h_sT_d), (gidx, gidx_d), (dst2d, dst_d), (cf2d, cf_d),
                    (dv2, dv2_d), (W1b, W1b_d), (W2b, W2b_d), (W1f, W1f_d),
                    (tW2f, tW2f_d), (tW1c, tW1c_d), (tb1c, tb1c_d),
                    (tb2c, tb2c_d), (tcol, tcol_d), (b1r, b1r_d), (b2r, b2r_d),
                    (lnwr, lnwr_d), (lnbr, lnbr_d), (iota, iota_d),
                    (linc, linc_d), (epsc, eps_d), (idmat, idm_d),
                    (onesr, ones_d),
                ]
                assert len(loads) == N_LOADS
                for dst_t, src_t in loads:
                    sync.dma_start(dst_t[:, :], src_t[:, :]).then_inc(s_ld, 16)

                for t in range(nt):
                    wt(sync, s_dv_pa, t + 1)
                    sync.dma_start(ag1_in[t * 128:(t + 1) * 128, :],
                                   xw1[:, t * 128:(t + 1) * 128]).then_inc(s_tb1, 16)

                if PHASE >= 4:
                    for w in range(nt):
                        wt(sync, s_dv_pa2, w + 1)
                        sync.dma_start(ag2_in[w * 128:(w + 1) * 128, :],
                                       xw2[:, w * 128:(w + 1) * 128]).then_inc(
                                           s_tb2, 16)

                for w in range(nt):
                    if PHASE == 1:
                        wt(sync, s_dv_pa, nt)
                    elif PHASE <= 4:
                        wt(sync, s_dv_drain, w + 1)
                    else:
                        wt(sync, s_dv_drain, nt + w + 1)
                    sync.dma_start(out_d[w * 128:(w + 1) * 128, :],
                                   agg[:, w * 128:(w + 1) * 128]).then_inc(s_out, 16)
                wt(sync, s_out, 16 * nt)

            # ---------------- GPSIMD ----------------
            @block.gpsimd
            def _(gpsimd):
                gpsimd.load_library(mlp)
                NCONV = 0 if PHASE == 1 else (1 if PHASE <= 4 else 2)
                for conv in range(NCONV):
                    wt(gpsimd, s_tb1 if conv == 0 else s_tb2, 16 * nt)
                    gpsimd.collective_compute(
                        "AllGather", Alu.bypass,
                        replica_groups=[list(range(N_CORES))],
                        ins=[ag_ins[conv].ap().opt()],
                        outs=[tables[conv].ap().opt()]).then_inc(s_cc, 1)
                    wt(gpsimd, s_cc, conv + 1)
                    maxcall = max(ncall_q) if ncall_q else 0
                    for j in range(maxcall):
                        for q in range(NBUK):
                            if j >= ncall_q[q]:
                                continue
                            if j >= NGBQ:
                                rel_w = call_rel_win[q][j - NGBQ]
                                wt(gpsimd, s_pe_run, conv * nt + rel_w + 1)
                            nchk = call_sizes[q][j]
                            nidx = nchk * 128
                            ec0 = buk_base[q] + j * CALL_CHUNKS
                            dstap = gb[:, q, j % NGBQ, 0:nchk * 128].rearrange(
                                "p (n e) -> p n e", e=128)
                            gpsimd.dma_gather(
                                dstap,
                                tables[conv][q * bukrows:(q + 1) * bukrows, :],
                                gidx[:, ec0 * 8:(ec0 + nchk) * 8], nidx, nidx,
                                128, single_packet=True, queue_num=q,
                            ).then_inc(s_gq[q], 16)
                for q in range(NBUK):
                    if ncall_q[q]:
                        wt(gpsimd, s_gq[q],
                           16 * (0 if PHASE == 1 else
                                 (1 if PHASE <= 4 else 2)) * ncall_q[q])

            # ---------------- TENSOR ----------------
            @block.tensor
            def _(tensor):
                wt(tensor, s_ld, 16 * N_LOADS)
                # t-MLP
                wt(tensor, s_dv_tm, 1)
                tensor.matmul(ps_pa[0][:, 0:1], tW2f[:, :], ucol[:, :],
                              start=True, stop=True).then_inc(s_pe_tm, 1)
                wt(tensor, s_dv_tm, 2)
                tensor.matmul(ps_pa[1][0:1, 0:128], vcol[:, :], W1f[:, :],
                              start=True, stop=True).then_inc(s_pe_tm, 1)
                wt(tensor, s_dv_tm, 3)
                # conv1 phase A
                for t in range(nt):
                    if t >= 2:
                        wt(tensor, s_dv_pa, t - 1)
                    p = ps_pa[t % 2]
                    tensor.matmul(p[:, :], hsT[:, t * 128:(t + 1) * 128], W1b[:, :],
                                  start=True, stop=False)
                    tensor.matmul(p[:, :], onesr[:, :], r1bf[:, :],
                                  start=False, stop=True).then_inc(s_pe_pa, 1)

                for conv in range(0 if PHASE == 1 else 1 if PHASE <= 4 else 2):
                    wt(tensor, s_dv_pa if conv == 0 else s_dv_pa2, nt)
                    xw = xw1 if conv == 0 else xw2
                    for w in range(nt):
                        W = conv * nt + w
                        if W >= NPS:
                            wt(tensor, s_dv_drain, W - NPS + 1)
                        i0 = win_item0[w]
                        i1 = win_item0[w + 1] if w + 1 < nt else nitem
                        wt(tensor, s_dv_oh, conv * nitem + i1)
                        for i in range(i0, i1):
                            it = items[i]
                            if it.kind == "edge":
                                wt(tensor, s_gq[it.q],
                                   16 * (conv * ncall_q[it.q] +
                                         it.pos // CALL_CHUNKS + 1))
                            lhs = oh[:, (conv * nitem + i) % NOH, :]
                            if it.kind == "diag":
                                rhs = xw[:, w * 128:(w + 1) * 128]
                            else:
                                slot = (it.pos // CALL_CHUNKS) % NGBQ
                                off = it.pos % CALL_CHUNKS
                                rhs = gb[:, it.q, slot, off * 128:(off + 1) * 128]
                            mm = tensor.matmul(ps_run[W % NPS][:, :], lhs, rhs,
                                               start=it.start, stop=it.stop)
                            if it.stop:
                                mm.then_inc(s_pe_run, 1)
                    if conv == 0 and PHASE >= 4:
                        def a2_mm(e):
                            wt(tensor, s_dv_trc, e + 1)
                            if e >= 2:
                                wt(tensor, s_dv_pa2, e - 1)
                            tensor.matmul(ps_pa[e % 2][:, :], h2T[:, e % 4, :],
                                          W2b[:, :], start=True,
                                          stop=True).then_inc(s_pe_pa2, 1)

                        for e in range(nt):
                            wt(tensor, s_ac_h2, e + 1)
                            if e >= 2:
                                wt(tensor, s_dv_trc, e - 1)
                            tensor.transpose(ps_tr[e % 2][:, :],
                                             h2full[:, e * 128:(e + 1) * 128],
                                             idmat[:, :]).then_inc(s_pe_tr, 1)
                            if e >= 1:
                                a2_mm(e - 1)
                        if nt >= 1:
                            a2_mm(nt - 1)

            # ---------------- VECTOR ----------------
            def emit_epilogue(vector, w):
                aggw = agg[:, w * 128:(w + 1) * 128]
                ssum, ssq = stat[:, 0:1], stat[:, 1:2]
                smu, ssmu = stat[:, 2:3], stat[:, 3:4]
                svarn, ssd, srstd = stat[:, 4:5], stat[:, 5:6], stat[:, 6:7]
                vector.drain()
                vector.tensor_reduce(ssum, aggw, AxisX, Alu.add)
                vector.tensor_mul(sqscr[:, :], aggw, aggw)
                vector.drain()
                vector.tensor_reduce(ssq, sqscr[:, :], AxisX, Alu.add)
                vector.drain()
                vector.tensor_scalar(smu, ssum, 1.0 / 128.0, None, Alu.mult)
                vector.drain()
                vector.tensor_scalar(ssmu, ssum, smu, None, Alu.mult)
                vector.drain()
                vector.tensor_scalar(svarn, ssq, ssmu, 1.0 / 128.0,
                                     Alu.subtract, Alu.mult).then_inc(s_dv_ep, 1)
                wt(vector, s_ac, w + 1)
                vector.reciprocal(srstd, ssd)
                vector.drain()
                if w >= 2:
                    wt(vector, s_ac_h2, w - 1)
                cw = c16[:, w % 2, :]
                vector.tensor_scalar(cw, aggw, smu, srstd, Alu.subtract, Alu.mult)
                vector.drain()
                vector.tensor_mul(cw, cw, lnwr[:, :])
                vector.drain()
                vector.tensor_add(cw, cw, lnbr[:, :]).then_inc(s_dv_ep, 1)

            @block.vector
            def _(vector):
                PH = PHASE
                wt(vector, s_ld, 16 * N_LOADS)
                # t-MLP
                vector.tensor_scalar(ucol[:, :], tW1c[:, :], tcol[:, :], tb1c[:, :],
                                     Alu.mult, Alu.add)
                vector.drain()
                vector.tensor_relu(ucol[:, :], ucol[:, :]).then_inc(s_dv_tm, 1)
                wt(vector, s_pe_tm, 1)
                vector.tensor_add(vcol[:, :], ps_pa[0][:, 0:1],
                                  tb2c[:, :]).then_inc(s_dv_tm, 1)
                wt(vector, s_pe_tm, 2)
                vector.tensor_copy(r1bf[:, :],
                                   ps_pa[1][0:1, 0:128]).then_inc(s_dv_tm, 1)
                # conv1 phase-A PSUM -> SBUF (bf16)
                for t in range(nt):
                    wt(vector, s_pe_pa, t + 1)
                    vector.tensor_copy(xw1[:, t * 128:(t + 1) * 128],
                                       ps_pa[t % 2][:, :]).then_inc(s_dv_pa, 1)

                def drain_win(vector, conv, dw, brep):
                    D = conv * nt + dw
                    aggw = agg[:, dw * 128:(dw + 1) * 128]
                    wt(vector, s_pe_run, D + 1)
                    vector.tensor_add(aggw, brep[:, :],
                                      ps_run[D % NPS][:, :]).then_inc(s_dv_drain, 1)
                    if conv == 0 and PH >= 3:
                        emit_epilogue(vector, dw)

                for conv in range(0 if PH == 1 else 1 if PH <= 4 else 2):
                    brep = b1r if conv == 0 else b2r
                    for w in range(nt):
                        i0 = win_item0[w]
                        i1 = win_item0[w + 1] if w + 1 < nt else nitem
                        for i in range(i0, i1):
                            gi = conv * nitem + i
                            it = items[i]
                            if gi >= NOH:
                                ii = gi - NOH
                                blk = (ii // nitem) * nt + items[ii % nitem].w
                                wt(vector, s_pe_run, blk + 1)
                            if it.kind == "diag":
                                s1, s2 = linc[:, :], dv2[:, w:w + 1]
                            else:
                                s1 = dst2d[:, it.ec:it.ec + 1]
                                s2 = cf2d[:, it.ec:it.ec + 1]
                            vector.tensor_scalar(oh[:, gi % NOH, :], iota[:, :],
                                                 s1, s2, Alu.is_equal,
                                                 Alu.mult).then_inc(s_dv_oh, 1)
                        if w >= 1:
                            drain_win(vector, conv, w - 1, brep)
                    drain_win(vector, conv, nt - 1, brep)
                    if conv == 0 and PH >= 4:
                        def pa2_copy(e):
                            wt(vector, s_pe_pa2, e + 1)
                            vector.tensor_copy(xw2[:, e * 128:(e + 1) * 128],
                                               ps_pa[e % 2][:, :]).then_inc(
                                                   s_dv_pa2, 1)

                        for e in range(nt):
                            wt(vector, s_pe_tr, e + 1)
                            if e >= 4:
                                wt(vector, s_pe_pa2, e - 3)
                            vector.tensor_copy(h2T[:, e % 4, :],
                                               ps_tr[e % 2][:, :]).then_inc(
                                                   s_dv_trc, 1)
                            if e >= 1:
                                pa2_copy(e - 1)
                        if nt >= 1:
                            pa2_copy(nt - 1)

            # ---------------- SCALAR (ACT) ----------------
            @block.scalar
            def _(scalar):
                wt(scalar, s_ld, 16 * N_LOADS)
                for e in range(nt if PHASE >= 3 else 0):
                    wt(scalar, s_dv_ep, 2 * e + 1)
                    scalar.activation(stat[:, 5:6], stat[:, 4:5], Act.Sqrt,
                                      bias=epsc[:, :]).then_inc(s_ac, 1)
                    wt(scalar, s_dv_ep, 2 * e + 2)
                    scalar.activation(h2full[:, e * 128:(e + 1) * 128],
                                      c16[:, e % 2, :],
                                      Act.Relu).then_inc(s_ac_h2, 1)

        nc.compile()
    return nc


# ---------------------------------------------------------------------------
# top level
# ---------------------------------------------------------------------------

LAST_NC = None


def _run_problem(h_noisy, edge_index, t, tW1, tb1, tW2, tb2, W1, b1, W2, b2,
                 ln_w, ln_b, n_nodes, shard, trace_dir=None):
    K = N_CORES
    npad = shard * K
    src = np.asarray(edge_index[0], np.int64)
    dst = np.asarray(edge_index[1], np.int64)

    deg = (np.bincount(dst, minlength=n_nodes).astype(np.float32) + 1.0)
    dinv = (1.0 / np.sqrt(deg)).astype(np.float32)
    coef = (dinv[src] * dinv[dst]).astype(np.float32)
    dinv2 = (dinv * dinv).astype(np.float32)
    dinv2_pad = np.ones(npad, np.float32)
    dinv2_pad[:n_nodes] = dinv2

    S = _make_schedule(src, dst, coef, shard)
    nt = S["nt"]

    h_pad = np.zeros((npad, C), np.float32)
    h_pad[:n_nodes] = np.asarray(h_noisy, np.float32)

    shared = {
        "W1b": np.asarray(W1, np.float32).astype(BF16),
        "W2b": np.asarray(W2, np.float32).astype(BF16),
        "W1f": np.asarray(W1, np.float32),
        "tW2f": np.asarray(tW2, np.float32),
        "tW1col": np.asarray(tW1, np.float32).reshape(C, 1),
        "tb1col": np.asarray(tb1, np.float32).reshape(C, 1),
        "tb2col": np.asarray(tb2, np.float32).reshape(C, 1),
        "tcol": np.full((C, 1), np.float32(np.asarray(t).reshape(-1)[0]), np.float32),
        "b1rep": np.tile(np.asarray(b1, np.float32).reshape(1, C), (128, 1)),
        "b2rep": np.tile(np.asarray(b2, np.float32).reshape(1, C), (128, 1)),
        "lnwrep": np.tile(np.asarray(ln_w, np.float32).reshape(1, C),
                          (128, 1)).astype(BF16),
        "lnbrep": np.tile(np.asarray(ln_b, np.float32).reshape(1, C),
                          (128, 1)).astype(BF16),
        "iota": np.tile(np.arange(128, dtype=np.float32), (128, 1)).astype(BF16),
        "lincol": np.arange(128, dtype=np.float32).reshape(128, 1),
        "epscol": np.full((128, 1), 1e-5, np.float32),
        "idmat": np.eye(128, dtype=np.float32).astype(BF16),
        "onesrow": np.ones((1, 128), np.float32).astype(BF16),
    }

    in_maps = []
    for k in range(K):
        gidx, dst2d, cf2d = S["core_arrays"][k]
        hs = h_pad[k * shard:(k + 1) * shard].astype(BF16)
        dv2col = np.zeros((128, nt), np.float32)
        for w in range(nt):
            dv2col[:, w] = dinv2_pad[k * shard + w * 128: k * shard + (w + 1) * 128]
        m = dict(shared)
        m["h_sT"] = np.ascontiguousarray(hs.T)
        m["gidx"] = gidx
        m["dst2d"] = dst2d
        m["coef2d"] = cf2d
        m["dinv2col"] = dv2col
        in_maps.append(m)

    nc = _build(S, shard)
    global LAST_NC
    LAST_NC = nc

    if trace_dir is not None:
        res = _run_traced(nc, in_maps, trace_dir)
    else:
        res = run_bass_kernel_spmd(nc, in_maps, list(range(K)))

    out = np.concatenate([res.results[k]["out_shard"] for k in range(K)], axis=0)
    return out[:n_nodes].astype(np.float32)


def _run_traced(nc, in_maps, trace_dir):
    """Run with NRT/NTFF profiling via the axon ctypes hook (test harness)."""
    import types
    import antenv
    if "antenv.axon_hooks" not in sys.modules:
        mod = types.ModuleType("antenv.axon_hooks")
        mod._hook = None
        mod.set_axon_ntff_profile_hook = lambda h: setattr(mod, "_hook", h)
        mod.get_axon_ntff_profile_hook = lambda: mod._hook
        sys.modules["antenv.axon_hooks"] = mod
        antenv.axon_hooks = mod
    from trn_agent_boot.trn_boot import _ntff_profile_via_ctypes
    hook = _ntff_profile_via_ctypes("/opt/axon/libaxon_pjrt.so")
    os.makedirs(trace_dir, exist_ok=True)
    with hook(trace_dir, [0]):
        res = run_bass_kernel_spmd(nc, in_maps, list(range(N_CORES)))
    return res


def kernel(h_noisy, edge_index, t, tW1, tb1, tW2, tb2, W1, b1, W2, b2,
           ln_w, ln_b):
    trace_dir = os.environ.get("BASS_KERNEL_TRACE_DIR") or None
    return _run_problem(
        np.asarray(h_noisy), np.asarray(edge_index), np.asarray(t),
        np.asarray(tW1), np.asarray(tb1), np.asarray(tW2), np.asarray(tb2),
        np.asarray(W1), np.asarray(b1), np.asarray(W2), np.asarray(b2),
        np.asarray(ln_w), np.asarray(ln_b),
        n_nodes=N_NODES, shard=12544, trace_dir=trace_dir)
